# revision 9
# baseline (speedup 1.0000x reference)
"""Multi-head causal attention (B=2, S=2048, D=1024, H=16, Dh=64) on 8 TRN2 cores.

Sharding: core = (b, g) with b = batch (2), g = head-group (4 heads each).
Each core computes QKV projections for its batch against its 4 heads' weight
columns, causal attention for those heads, and the partial output projection
against its 4 heads' Wo rows.  Host sums the 4 partials per batch and adds
the bias.

Precision: bf16 matmuls with fp32 PSUM accumulation everywhere EXCEPT the
score matmuls, which store Q^T/K^T in fp8 (e4m3) and run in DoubleRow perf
mode: lhsT/rhs carry a stride-0 broadcast pair so one 0.5-cycle/row DR pass
contracts dh=64 twice (the doubled scores fold into the softmax exp scale
1/16).  fp8 elsewhere fails the 2e-2 gate: per-element quantization noise
(~2.7% for e4m3) passes through dot products against random data undamped,
and the independent contributions stack to ~5.5e-2.

Layouts avoid all on-chip transposes:
  x^T [128, 8k, S] k-tile-major feeds projections directly
  V is projected in [s, dh] orientation (x^T tiles as lhsT), landing
  AV-ready with an appended ones column (row 64 accumulates softmax sums)
  scores are computed transposed [k, q] so exp output feeds AV directly

Engine split: PE does matmuls only; ACT does exp only; DVE handles
PSUM-sourced copies/reciprocals and the normalization multiply; the
otherwise-idle GPSIMD does the causal staircase mask multiplies and the
1/sums partition broadcast (replacing the baseline's rank-1 PE matmuls).
"""

import numpy as np
import ml_dtypes

B = 2
S = 2048
D = 1024
HPC = 4  # heads per core
DH = 64
QB = 512  # q band width
NB = S // QB  # 4 bands
KT = 128  # k tile
N_CORES = 8

# exp(s_psum * EXP_SCALE) = exp(s_true / sqrt(DH)); the stride-0 DR pair
# doubles s_psum.
EXP_SCALE = 1.0 / 16.0

_CACHE = {}


def _build_bass():
    import concourse.bacc as bacc
    import concourse.tile as tile
    from concourse import mybir

    f32 = mybir.dt.float32
    bf16 = mybir.dt.bfloat16
    fp8 = mybir.dt.float8e4
    DR = mybir.MatmulPerfMode.DoubleRow
    ExpF = mybir.ActivationFunctionType.Exp

    nc = bacc.Bacc("TRN2", target_bir_lowering=False)

    xT_d = nc.dram_tensor("xT", [128, 8, S], bf16, kind="ExternalInput")
    wqkv_d = nc.dram_tensor("wqkv", [128, 8, 768], bf16, kind="ExternalInput")
    wo_d = nc.dram_tensor("wo", [128, 2, D], bf16, kind="ExternalInput")
    masks_d = nc.dram_tensor("masks", [128, 4, QB], bf16, kind="ExternalInput")
    out_d = nc.dram_tensor("out", [S, D], bf16, kind="ExternalOutput")

    with tile.TileContext(nc) as tc:
        with (
            tc.tile_pool(name="consts", bufs=1) as consts,
            tc.tile_pool(name="persist", bufs=1) as persist,
            tc.tile_pool(name="score_ps", bufs=2, space="PSUM") as score_ps,
            tc.tile_pool(name="ctx_ps", bufs=2, space="PSUM") as ctx_ps,
            tc.tile_pool(name="misc_ps", bufs=2, space="PSUM") as misc_ps,
            tc.tile_pool(name="at_pool", bufs=8) as at_pool,
            tc.tile_pool(name="rr_pool", bufs=4) as rr_pool,
            tc.tile_pool(name="rb_pool", bufs=4) as rb_pool,
            tc.tile_pool(name="osb_pool", bufs=6) as osb_pool,
        ):
            # ---- constants: weights first (first proj group needs them),
            #      band-0 x^T slices, masks; later x^T bands stream behind ----
            wqkv = consts.tile([128, 8, 768], bf16, tag="wqkv", name="wqkv")
            xT = consts.tile([128, 8, S], bf16, tag="xT", name="xT")
            nc.sync.dma_start(out=wqkv[:, 0, :], in_=wqkv_d[:, 0, :])
            nc.sync.dma_start(out=xT[:, 0:4, 0:QB], in_=xT_d[:, 0:4, 0:QB])
            nc.sync.dma_start(out=xT[:, 4:8, 0:QB], in_=xT_d[:, 4:8, 0:QB])
            nc.sync.dma_start(out=wqkv[:, 1:8, :], in_=wqkv_d[:, 1:8, :])
            for j in range(1, NB):
                nc.sync.dma_start(
                    out=xT[:, :, j * QB : (j + 1) * QB],
                    in_=xT_d[:, :, j * QB : (j + 1) * QB],
                )
            mask_sb = consts.tile([128, 4, QB], bf16, tag="masks", name="masks")
            nc.sync.dma_start(out=mask_sb, in_=masks_d[:, :, :])
            wo = consts.tile([128, 2, D], bf16, tag="wo", name="wo")
            nc.sync.dma_start(out=wo, in_=wo_d[:, :, :])

            # ---- persistent activations ----
            qT = [
                persist.tile([128, S], fp8, tag=f"qT{p}", name=f"qT{p}")
                for p in range(2)
            ]
            kTt = [
                persist.tile([128, S], fp8, tag=f"kT{p}", name=f"kT{p}")
                for p in range(2)
            ]
            # v: (k-position, k-tile, head-in-pair, dh + ones column)
            vp = [
                persist.tile([128, 16, 2, 65], bf16, tag=f"vp{p}", name=f"vp{p}")
                for p in range(2)
            ]
            # ctx^T, normalized: (dh-in-pair, pair, q)
            ctxo = persist.tile([128, 2, S], bf16, tag="ctxo", name="ctxo")
            for p in range(2):
                nc.gpsimd.memset(vp[p][:, :, :, 64:65], 1.0)

            def dr2(ap, n):
                """View a [64, n] slice as a stride-0 [64, 2, n] DR pair."""
                return ap.unsqueeze(1).broadcast_to([64, 2, n])

            filler_q = []  # (est_ns, tag, closure) independent PE chains

            def emit_qk_chain(t, dest, p, j, h):
                q0 = j * QB + h * (QB // 2)
                c0 = 256 * t + 128 * p
                ps = misc_ps.tile([128, QB // 2], f32, tag="misc", name="pqk")
                for k in range(8):
                    nc.tensor.matmul(
                        ps,
                        lhsT=wqkv[:, k, c0 : c0 + 128],
                        rhs=xT[:, k, q0 : q0 + QB // 2],
                        start=(k == 0),
                        stop=(k == 7),
                    )
                nc.vector.tensor_copy(
                    out=dest[p][:, q0 : q0 + QB // 2], in_=ps
                )

            def emit_v_chain(kt, p):
                c0 = 512 + 128 * p
                ps = misc_ps.tile([128, 2, 64], f32, tag="misc", name="pv")
                for k in range(8):
                    nc.tensor.matmul(
                        ps,
                        lhsT=xT[:, k, kt * KT : (kt + 1) * KT],
                        rhs=wqkv[:, k, c0 : c0 + 128],
                        start=(k == 0),
                        stop=(k == 7),
                    )
                nc.vector.tensor_copy(out=vp[p][:, kt, :, 0:64], in_=ps)

            def emit_proj(j):
                """QKV projections for band j (bf16, fp32 PSUM).

                Q^T/K^T land as fp8 [128, QB] slabs (pair rows = 2 heads x
                64 dh) feeding the DR score matmuls.  V is projected
                directly in [s, dh] orientation (x^T tiles as lhsT), so no
                on-chip transposes are needed."""
                for t, dest in ((0, qT), (1, kTt)):
                    for p in range(2):
                        for h in range(2):
                            emit_qk_chain(t, dest, p, j, h)
                for kt4 in range(4):
                    for p in range(2):
                        emit_v_chain(4 * j + kt4, p)

            def queue_proj(j):
                for t, dest in ((0, qT), (1, kTt)):
                    for p in range(2):
                        for h in range(2):
                            filler_q.append(
                                (860, ("proj", j),
                                 lambda t=t, dest=dest, p=p, h=h:
                                     emit_qk_chain(t, dest, p, j, h))
                            )
                for kt4 in range(4):
                    for p in range(2):
                        filler_q.append(
                            (430, ("proj", j),
                             lambda kt=4 * j + kt4, p=p: emit_v_chain(kt, p))
                        )

            def emit_op_chain(j, m, n, last):
                NQ = QB // 2
                ops = misc_ps.tile([128, NQ], f32, tag="misc", name="ops")
                for p in range(2):
                    nc.tensor.matmul(
                        ops,
                        lhsT=ctxo[:, p, m * KT : (m + 1) * KT],
                        rhs=wo[:, p, n * NQ : (n + 1) * NQ],
                        start=(p == 0),
                        stop=(p == 1),
                    )
                osb = osb_q[m]
                nc.vector.tensor_copy(out=osb[:, n * NQ : (n + 1) * NQ], in_=ops)
                if last:
                    nc.sync.dma_start(
                        out=out_d[m * KT : (m + 1) * KT, n * NQ : (n + 1) * NQ],
                        in_=osb[:, n * NQ : (n + 1) * NQ],
                    )
                elif n == 3:
                    nc.sync.dma_start(
                        out=out_d[m * KT : (m + 1) * KT, :], in_=osb
                    )

            osb_q = {}

            def queue_outproj(j):
                last = j == NB - 1
                for m in range(4 * j, 4 * j + 4):
                    osb_q[m] = osb_pool.tile([128, D], bf16, tag="osb", name="osb")
                    for n in range(4):
                        filler_q.append(
                            (215, ("outproj", j),
                             lambda m=m, n=n: emit_op_chain(j, m, n, last))
                        )

            bal = {"act": 0.0, "pe": 0.0}

            def drain_fillers(tag_proj_band=None, all_=False):
                """Emit queued chains: FIFO through the last must-emit item
                (band j's projection matmuls must precede band j's score
                matmuls in the in-order PE stream), then keep filling while
                the ACT-time estimate leads the PE one."""

                while deferred_mults:
                    deferred_mults.pop(0)()

                def is_must(e):
                    kind, b = e[1]
                    return all_ or (
                        kind == "proj"
                        and tag_proj_band is not None
                        and b <= tag_proj_band
                    )

                while any(is_must(e) for e in filler_q):
                    est, _, cl = filler_q.pop(0)
                    cl()
                    bal["pe"] += est
                while filler_q and bal["act"] > bal["pe"]:
                    est, _, cl = filler_q.pop(0)
                    cl()
                    bal["pe"] += est

            deferred_mults = []

            def emit_norm_pc(j, p, c, cps):
                """ctx rows / softmax sums (ctx PSUM row 64): DVE reciprocal
                of the sums row, GPSIMD broadcast across partitions, one DVE
                multiply writing the normalized bf16 ctx^T operand.  The
                multiply is deferred one filler-drain cycle so it never
                head-of-line-blocks the DVE queue waiting on the Pool
                broadcast."""
                q0 = j * QB
                rr = rr_pool.tile([1, QB], bf16, tag="rr", name="rr")
                with nc.allow_low_precision(
                    reason="reciprocal feeds a bf16 multiply"
                ):
                    nc.vector.reciprocal(out=rr, in_=cps[64:65, :])
                rbs = rb_pool.tile([64, QB], bf16, tag="rb", name="rb")
                nc.gpsimd.partition_broadcast(rbs, rr)
                deferred_mults.append(
                    lambda: nc.vector.tensor_mul(
                        ctxo[64 * c : 64 * c + 64, p, q0 : q0 + QB],
                        cps[0:64, :],
                        rbs,
                    )
                )

            def emit_attention(j):
                """Scores+softmax+AV for band j.

                Scores land transposed ([k, q]) in a [128, 2, QB] fp32 PSUM
                tile per (pair, k-tile-pair, head); one exp covers both
                halves.  Diagonal pairs extend the odd k-tile's q-range down
                to the even tile's start so the exp stays a single strided
                instruction; the AV matmuls read per-tile causal ranges so
                the extension region is never consumed.  GPSIMD applies the
                128-wide staircase mask strips after exp.

                Per (pair, k-pair) group the PE stream is: both heads'
                score matmuls, then queued filler chains sized to the
                ACT-vs-PE balance (the exp is ~3x the group's matmul time),
                then both heads' AV matmuls."""
                q0 = j * QB
                n_i2 = 2 * (j + 1)
                for p in range(2):
                    cps = [
                        ctx_ps.tile([65, QB], f32, tag="ctx", name="ctx")
                        for _ in range(2)
                    ]
                    for i2 in range(n_i2):
                        o_e = 2 * i2 - 4 * j
                        diag = o_e >= 0
                        z_e = 128 * o_e if diag else 0
                        z_o = z_e + 128 if diag else 0
                        sps_c, at_c = [], []
                        for c in range(2):
                            sps = score_ps.tile(
                                [128, 2, QB], f32, tag="sps", name="sps"
                            )
                            for half in range(2):
                                i = 2 * i2 + half
                                nc.tensor.matmul(
                                    sps[:, half, z_e:QB],
                                    lhsT=dr2(
                                        kTt[p][
                                            64 * c : 64 * c + 64,
                                            i * KT : (i + 1) * KT,
                                        ],
                                        KT,
                                    ),
                                    rhs=dr2(
                                        qT[p][
                                            64 * c : 64 * c + 64,
                                            q0 + z_e : q0 + QB,
                                        ],
                                        QB - z_e,
                                    ),
                                    start=True,
                                    stop=True,
                                    perf_mode=DR,
                                )
                            sps_c.append(sps)
                            bal["pe"] += 2 * (QB - z_e) * 0.5 * 0.42
                        for c in range(2):
                            at = at_pool.tile(
                                [128, 2, QB], bf16, tag="at", name="at"
                            )
                            nc.scalar.activation(
                                out=at[:, :, z_e:QB],
                                in_=sps_c[c][:, :, z_e:QB],
                                func=ExpF,
                                scale=EXP_SCALE,
                            )
                            if diag:
                                nc.gpsimd.tensor_mul(
                                    at[:, 0, z_e:z_o],
                                    at[:, 0, z_e:z_o],
                                    mask_sb[:, o_e, z_e:z_o],
                                )
                                nc.gpsimd.tensor_mul(
                                    at[:, 1, z_o : z_o + 128],
                                    at[:, 1, z_o : z_o + 128],
                                    mask_sb[:, o_e + 1, z_o : z_o + 128],
                                )
                            at_c.append(at)
                            bal["act"] += 2 * (QB - z_e) * 0.833 + 185
                        drain_fillers()
                        for c in range(2):
                            for half in range(2):
                                i = 2 * i2 + half
                                o = i - 4 * j
                                z = 128 * o if o > 0 else 0
                                nc.tensor.matmul(
                                    cps[c][:, z:QB],
                                    lhsT=vp[p][:, i, c, :],
                                    rhs=at_c[c][:, half, z:QB],
                                    start=(i == 0),
                                    stop=(i == 4 * (j + 1) - 1),
                                )
                                bal["pe"] += (QB - z) * 0.42
                    for c in range(2):
                        emit_norm_pc(j, p, c, cps[c])

            # band-major pipeline with filler interleaving: the next
            # band's projection chains and the previous band's output
            # projection fill the PE bubbles while exp (ACT) works through
            # the current band's score tiles.
            emit_proj(0)
            for j in range(NB):
                if j + 1 < NB:
                    queue_proj(j + 1)
                emit_attention(j)
                drain_fillers(tag_proj_band=j + 1)
                queue_outproj(j)
            drain_fillers(all_=True)

    nc.compile()
    return nc


def _get_bass():
    if "nc" not in _CACHE:
        _CACHE["nc"] = _build_bass()
    return _CACHE["nc"]


def _make_in_maps(x, Wq, Wk, Wv, Wo):
    bf = ml_dtypes.bfloat16
    if "masks" not in _CACHE:
        # causal staircase masks: keep iff q >= k + 128*o  (within a band, a
        # k-tile at offset o*128 above the band start)
        kp = np.arange(128)[:, None]
        qf = np.arange(QB)[None, :]
        _CACHE["masks"] = np.ascontiguousarray(
            np.stack(
                [(qf >= kp + 128 * o).astype(np.float32) for o in range(4)]
            ).transpose(1, 0, 2)
        ).astype(bf)
    masks = _CACHE["masks"]

    # x^T in k-tile-major layout: (p, k, s) = x[b][s, 128k + p]
    xTs = [
        np.ascontiguousarray(
            x[b].T.reshape(8, 128, S).transpose(1, 0, 2)
        ).astype(bf)
        for b in range(B)
    ]
    in_maps = []
    for core in range(N_CORES):
        b, g = divmod(core, 4)
        hs = slice(g * 256, (g + 1) * 256)
        if core < 4:
            wqkv_f = np.concatenate([Wq[:, hs], Wk[:, hs], Wv[:, hs]], axis=1)
            shards = {
                "wqkv": np.ascontiguousarray(
                    wqkv_f.reshape(8, 128, 768).transpose(1, 0, 2)
                ).astype(bf),
                "wo": np.ascontiguousarray(
                    Wo[hs, :].reshape(2, 128, D).transpose(1, 0, 2)
                ).astype(bf),
            }
        else:
            shards = {k: in_maps[core - 4][k] for k in ("wqkv", "wo")}
        in_maps.append({"xT": xTs[b], "masks": masks, **shards})
    return in_maps


def _run(x, Wq, Wk, Wv, Wo, bo, trace=False):
    from concourse.bass_utils import run_bass_kernel_spmd

    nc = _get_bass()
    in_maps = _make_in_maps(x, Wq, Wk, Wv, Wo)
    res = run_bass_kernel_spmd(
        nc, in_maps, core_ids=list(range(N_CORES)), trace=trace
    )
    out = np.zeros((B, S, D), np.float32)
    for core in range(N_CORES):
        out[core // 4] += res.results[core]["out"].astype(np.float32)
    out += bo.astype(np.float32)
    return out, res


def kernel(x, Wq, Wk, Wv, Wo, bo):
    x, Wq, Wk, Wv, Wo, bo = (np.asarray(a) for a in (x, Wq, Wk, Wv, Wo, bo))
    out, _ = _run(x, Wq, Wk, Wv, Wo, bo, trace=False)
    return out


def kernel_traced(x, Wq, Wk, Wv, Wo, bo):
    """Same as kernel() but returns (out, BassKernelResults) with profiling."""
    x, Wq, Wk, Wv, Wo, bo = (np.asarray(a) for a in (x, Wq, Wk, Wv, Wo, bo))
    return _run(x, Wq, Wk, Wv, Wo, bo, trace=True)


# revision 10
# speedup vs baseline: 1.0299x; 1.0299x over previous
"""Multi-head causal attention (B=2, S=2048, D=1024, H=16, Dh=64) on 8 TRN2 cores.

Sharding: core = (b, g) with b = batch (2), g = head-group (4 heads each).
Each core computes QKV projections for its batch against its 4 heads' weight
columns, causal attention for those heads, and the partial output projection
against its 4 heads' Wo rows.  Host sums the 4 partials per batch and adds
the bias.

Precision: bf16 matmuls with fp32 PSUM accumulation everywhere EXCEPT the
score matmuls, which store Q^T/K^T in fp8 (e4m3) and run in DoubleRow perf
mode: lhsT/rhs carry a stride-0 broadcast pair so one 0.5-cycle/row DR pass
contracts dh=64 twice (the doubled scores fold into the softmax exp scale
1/16).  fp8 elsewhere fails the 2e-2 gate: per-element quantization noise
(~2.7% for e4m3) passes through dot products against random data undamped,
and the independent contributions stack to ~5.5e-2.

Layouts avoid all on-chip transposes:
  x^T [128, 8k, S] k-tile-major feeds projections directly
  V is projected in [s, dh] orientation (x^T tiles as lhsT), landing
  AV-ready with an appended ones column (row 64 accumulates softmax sums)
  scores are computed transposed [k, q] so exp output feeds AV directly

Engine split: PE does matmuls only; ACT does exp only; DVE handles
PSUM-sourced copies/reciprocals and the normalization multiply; the
otherwise-idle GPSIMD does the causal staircase mask multiplies and the
1/sums partition broadcast (replacing the baseline's rank-1 PE matmuls).
"""

import numpy as np
import ml_dtypes

B = 2
S = 2048
D = 1024
HPC = 4  # heads per core
DH = 64
QB = 512  # q band width
NB = S // QB  # 4 bands
KT = 128  # k tile
N_CORES = 8

# exp(s_psum * EXP_SCALE) = exp(s_true / sqrt(DH)); the stride-0 DR pair
# doubles s_psum.
EXP_SCALE = 1.0 / 16.0

_CACHE = {}


def _build_bass():
    import concourse.bacc as bacc
    import concourse.tile as tile
    from concourse import mybir

    f32 = mybir.dt.float32
    bf16 = mybir.dt.bfloat16
    fp8 = mybir.dt.float8e4
    DR = mybir.MatmulPerfMode.DoubleRow
    ExpF = mybir.ActivationFunctionType.Exp

    nc = bacc.Bacc("TRN2", target_bir_lowering=False)

    xT_d = nc.dram_tensor("xT", [128, 8, S], bf16, kind="ExternalInput")
    wqkv_d = nc.dram_tensor("wqkv", [128, 8, 768], bf16, kind="ExternalInput")
    wo_d = nc.dram_tensor("wo", [128, 2, D], bf16, kind="ExternalInput")
    masks_d = nc.dram_tensor("masks", [128, 4, QB], bf16, kind="ExternalInput")
    out_d = nc.dram_tensor("out", [S, D], bf16, kind="ExternalOutput")

    with tile.TileContext(nc) as tc:
        with (
            tc.tile_pool(name="consts", bufs=1) as consts,
            tc.tile_pool(name="persist", bufs=1) as persist,
            tc.tile_pool(name="score_ps", bufs=2, space="PSUM") as score_ps,
            tc.tile_pool(name="ctx_ps", bufs=2, space="PSUM") as ctx_ps,
            tc.tile_pool(name="misc_ps", bufs=2, space="PSUM") as misc_ps,
            tc.tile_pool(name="at_pool", bufs=8) as at_pool,
            tc.tile_pool(name="rr_pool", bufs=4) as rr_pool,
            tc.tile_pool(name="rb_pool", bufs=4) as rb_pool,
            tc.tile_pool(name="osb_pool", bufs=6) as osb_pool,
        ):
            # ---- constants: weights first (first proj group needs them),
            #      band-0 x^T slices, masks; later x^T bands stream behind ----
            wqkv = consts.tile([128, 8, 768], bf16, tag="wqkv", name="wqkv")
            xT = consts.tile([128, 8, S], bf16, tag="xT", name="xT")
            nc.sync.dma_start(out=wqkv[:, 0, :], in_=wqkv_d[:, 0, :])
            nc.sync.dma_start(out=xT[:, 0:4, 0:QB], in_=xT_d[:, 0:4, 0:QB])
            for k in range(1, 4):
                nc.sync.dma_start(out=wqkv[:, k, :], in_=wqkv_d[:, k, :])
            nc.sync.dma_start(out=xT[:, 4:8, 0:QB], in_=xT_d[:, 4:8, 0:QB])
            for k in range(4, 8):
                nc.sync.dma_start(out=wqkv[:, k, :], in_=wqkv_d[:, k, :])
            for j in range(1, NB):
                nc.sync.dma_start(
                    out=xT[:, :, j * QB : (j + 1) * QB],
                    in_=xT_d[:, :, j * QB : (j + 1) * QB],
                )
            mask_sb = consts.tile([128, 4, QB], bf16, tag="masks", name="masks")
            nc.sync.dma_start(out=mask_sb, in_=masks_d[:, :, :])
            wo = consts.tile([128, 2, D], bf16, tag="wo", name="wo")
            nc.sync.dma_start(out=wo, in_=wo_d[:, :, :])

            # ---- persistent activations ----
            qT = [
                persist.tile([128, S], fp8, tag=f"qT{p}", name=f"qT{p}")
                for p in range(2)
            ]
            kTt = [
                persist.tile([128, S], fp8, tag=f"kT{p}", name=f"kT{p}")
                for p in range(2)
            ]
            # v: (k-position, k-tile, head-in-pair, dh + ones column)
            vp = [
                persist.tile([128, 16, 2, 65], bf16, tag=f"vp{p}", name=f"vp{p}")
                for p in range(2)
            ]
            # ctx^T, normalized: (dh-in-pair, pair, q)
            ctxo = persist.tile([128, 2, S], bf16, tag="ctxo", name="ctxo")
            for p in range(2):
                nc.gpsimd.memset(vp[p][:, :, :, 64:65], 1.0)

            def dr2(ap, n):
                """View a [64, n] slice as a stride-0 [64, 2, n] DR pair."""
                return ap.unsqueeze(1).broadcast_to([64, 2, n])

            filler_q = []  # (est_ns, tag, closure) independent PE chains

            def emit_qk_chain(t, dest, p, j, h):
                q0 = j * QB + h * (QB // 2)
                c0 = 256 * t + 128 * p
                ps = misc_ps.tile([128, QB // 2], f32, tag="misc", name="pqk")
                for k in range(8):
                    nc.tensor.matmul(
                        ps,
                        lhsT=wqkv[:, k, c0 : c0 + 128],
                        rhs=xT[:, k, q0 : q0 + QB // 2],
                        start=(k == 0),
                        stop=(k == 7),
                    )
                nc.vector.tensor_copy(
                    out=dest[p][:, q0 : q0 + QB // 2], in_=ps
                )

            def emit_v_chain(kt, p):
                c0 = 512 + 128 * p
                ps = misc_ps.tile([128, 2, 64], f32, tag="misc", name="pv")
                for k in range(8):
                    nc.tensor.matmul(
                        ps,
                        lhsT=xT[:, k, kt * KT : (kt + 1) * KT],
                        rhs=wqkv[:, k, c0 : c0 + 128],
                        start=(k == 0),
                        stop=(k == 7),
                    )
                nc.vector.tensor_copy(out=vp[p][:, kt, :, 0:64], in_=ps)

            def emit_proj(j):
                """QKV projections for band j (bf16, fp32 PSUM).

                Q^T/K^T land as fp8 [128, QB] slabs (pair rows = 2 heads x
                64 dh) feeding the DR score matmuls.  V is projected
                directly in [s, dh] orientation (x^T tiles as lhsT), so no
                on-chip transposes are needed."""
                for t, dest in ((0, qT), (1, kTt)):
                    for p in range(2):
                        for h in range(2):
                            emit_qk_chain(t, dest, p, j, h)
                for kt4 in range(4):
                    for p in range(2):
                        emit_v_chain(4 * j + kt4, p)

            def queue_proj(j):
                for t, dest in ((0, qT), (1, kTt)):
                    for p in range(2):
                        for h in range(2):
                            filler_q.append(
                                (860, ("proj", j),
                                 lambda t=t, dest=dest, p=p, h=h:
                                     emit_qk_chain(t, dest, p, j, h))
                            )
                for kt4 in range(4):
                    for p in range(2):
                        filler_q.append(
                            (430, ("proj", j),
                             lambda kt=4 * j + kt4, p=p: emit_v_chain(kt, p))
                        )

            def emit_op_chain(j, m, n, last):
                NQ = QB // 2
                ops = misc_ps.tile([128, NQ], f32, tag="misc", name="ops")
                for p in range(2):
                    nc.tensor.matmul(
                        ops,
                        lhsT=ctxo[:, p, m * KT : (m + 1) * KT],
                        rhs=wo[:, p, n * NQ : (n + 1) * NQ],
                        start=(p == 0),
                        stop=(p == 1),
                    )
                osb = osb_q[m]
                nc.vector.tensor_copy(out=osb[:, n * NQ : (n + 1) * NQ], in_=ops)
                if last:
                    nc.sync.dma_start(
                        out=out_d[m * KT : (m + 1) * KT, n * NQ : (n + 1) * NQ],
                        in_=osb[:, n * NQ : (n + 1) * NQ],
                    )
                elif n == 3:
                    nc.sync.dma_start(
                        out=out_d[m * KT : (m + 1) * KT, :], in_=osb
                    )

            osb_q = {}

            def queue_outproj(j):
                last = j == NB - 1
                for m in range(4 * j, 4 * j + 4):
                    osb_q[m] = osb_pool.tile([128, D], bf16, tag="osb", name="osb")
                    for n in range(4):
                        filler_q.append(
                            (215, ("outproj", j),
                             lambda m=m, n=n: emit_op_chain(j, m, n, last))
                        )

            bal = {"act": 0.0, "pe": 0.0}

            def drain_fillers(tag_proj_band=None, all_=False):
                """Emit queued chains: FIFO through the last must-emit item
                (band j's projection matmuls must precede band j's score
                matmuls in the in-order PE stream), then keep filling while
                the ACT-time estimate leads the PE one."""

                def is_must(e):
                    kind, b = e[1]
                    return all_ or (
                        kind == "proj"
                        and tag_proj_band is not None
                        and b <= tag_proj_band
                    )

                while any(is_must(e) for e in filler_q):
                    est, _, cl = filler_q.pop(0)
                    cl()
                    bal["pe"] += est
                while filler_q and bal["act"] > bal["pe"]:
                    est, _, cl = filler_q.pop(0)
                    cl()
                    bal["pe"] += est

            def emit_norm_pc(j, p, c, cps):
                """ctx rows / softmax sums (ctx PSUM row 64): DVE does the
                dependency-free PSUM reads (reciprocal of the sums row, ctx
                copy to SBUF -- releasing the ctx PSUM bank early); GPSIMD
                broadcasts 1/sums across partitions and applies the
                all-SBUF normalization multiply, keeping the cross-engine
                wait off the DVE queue (whose copies release the PE's
                PSUM slots)."""
                q0 = j * QB
                rr = rr_pool.tile([1, QB], bf16, tag="rr", name="rr")
                with nc.allow_low_precision(
                    reason="reciprocal feeds a bf16 multiply"
                ):
                    nc.vector.reciprocal(out=rr, in_=cps[64:65, :])
                cf = rb_pool.tile([64, QB], bf16, tag="cf", name="cf")
                nc.vector.tensor_copy(out=cf, in_=cps[0:64, :])
                rbs = rb_pool.tile([64, QB], bf16, tag="rb", name="rb")
                nc.gpsimd.partition_broadcast(rbs, rr)
                nc.gpsimd.tensor_mul(
                    ctxo[64 * c : 64 * c + 64, p, q0 : q0 + QB],
                    cf,
                    rbs,
                )

            def emit_attention(j):
                """Scores+softmax+AV for band j.

                Scores land transposed ([k, q]) in a [128, 2, QB] fp32 PSUM
                tile per (pair, k-tile-pair, head); one exp covers both
                halves.  Diagonal pairs extend the odd k-tile's q-range down
                to the even tile's start so the exp stays a single strided
                instruction; the AV matmuls read per-tile causal ranges so
                the extension region is never consumed.  GPSIMD applies the
                128-wide staircase mask strips after exp.

                Per (pair, k-pair) group the PE stream is: both heads'
                score matmuls, then queued filler chains sized to the
                ACT-vs-PE balance (the exp is ~3x the group's matmul time),
                then both heads' AV matmuls."""
                q0 = j * QB
                n_i2 = 2 * (j + 1)
                for p in range(2):
                    cps = [
                        ctx_ps.tile([65, QB], f32, tag="ctx", name="ctx")
                        for _ in range(2)
                    ]
                    for i2 in range(n_i2):
                        o_e = 2 * i2 - 4 * j
                        diag = o_e >= 0
                        z_e = 128 * o_e if diag else 0
                        z_o = z_e + 128 if diag else 0
                        sps_c, at_c = [], []
                        for c in range(2):
                            sps = score_ps.tile(
                                [128, 2, QB], f32, tag="sps", name="sps"
                            )
                            for half in range(2):
                                i = 2 * i2 + half
                                nc.tensor.matmul(
                                    sps[:, half, z_e:QB],
                                    lhsT=dr2(
                                        kTt[p][
                                            64 * c : 64 * c + 64,
                                            i * KT : (i + 1) * KT,
                                        ],
                                        KT,
                                    ),
                                    rhs=dr2(
                                        qT[p][
                                            64 * c : 64 * c + 64,
                                            q0 + z_e : q0 + QB,
                                        ],
                                        QB - z_e,
                                    ),
                                    start=True,
                                    stop=True,
                                    perf_mode=DR,
                                )
                            sps_c.append(sps)
                            bal["pe"] += 2 * (QB - z_e) * 0.5 * 0.42
                        for c in range(2):
                            at = at_pool.tile(
                                [128, 2, QB], bf16, tag="at", name="at"
                            )
                            nc.scalar.activation(
                                out=at[:, :, z_e:QB],
                                in_=sps_c[c][:, :, z_e:QB],
                                func=ExpF,
                                scale=EXP_SCALE,
                            )
                            if diag:
                                nc.gpsimd.tensor_mul(
                                    at[:, 0, z_e:z_o],
                                    at[:, 0, z_e:z_o],
                                    mask_sb[:, o_e, z_e:z_o],
                                )
                                nc.gpsimd.tensor_mul(
                                    at[:, 1, z_o : z_o + 128],
                                    at[:, 1, z_o : z_o + 128],
                                    mask_sb[:, o_e + 1, z_o : z_o + 128],
                                )
                            at_c.append(at)
                            bal["act"] += 2 * (QB - z_e) * 0.833 + 185
                        drain_fillers()
                        for c in range(2):
                            for half in range(2):
                                i = 2 * i2 + half
                                o = i - 4 * j
                                z = 128 * o if o > 0 else 0
                                nc.tensor.matmul(
                                    cps[c][:, z:QB],
                                    lhsT=vp[p][:, i, c, :],
                                    rhs=at_c[c][:, half, z:QB],
                                    start=(i == 0),
                                    stop=(i == 4 * (j + 1) - 1),
                                )
                                bal["pe"] += (QB - z) * 0.42
                    for c in range(2):
                        emit_norm_pc(j, p, c, cps[c])

            # band-major pipeline with filler interleaving: the next
            # band's projection chains and the previous band's output
            # projection fill the PE bubbles while exp (ACT) works through
            # the current band's score tiles.
            emit_proj(0)
            for j in range(NB):
                if j + 1 < NB:
                    queue_proj(j + 1)
                emit_attention(j)
                drain_fillers(tag_proj_band=j + 1)
                queue_outproj(j)
            drain_fillers(all_=True)

    nc.compile()
    return nc


def _get_bass():
    if "nc" not in _CACHE:
        _CACHE["nc"] = _build_bass()
    return _CACHE["nc"]


def _make_in_maps(x, Wq, Wk, Wv, Wo):
    bf = ml_dtypes.bfloat16
    if "masks" not in _CACHE:
        # causal staircase masks: keep iff q >= k + 128*o  (within a band, a
        # k-tile at offset o*128 above the band start)
        kp = np.arange(128)[:, None]
        qf = np.arange(QB)[None, :]
        _CACHE["masks"] = np.ascontiguousarray(
            np.stack(
                [(qf >= kp + 128 * o).astype(np.float32) for o in range(4)]
            ).transpose(1, 0, 2)
        ).astype(bf)
    masks = _CACHE["masks"]

    # x^T in k-tile-major layout: (p, k, s) = x[b][s, 128k + p]
    xTs = [
        np.ascontiguousarray(
            x[b].T.reshape(8, 128, S).transpose(1, 0, 2)
        ).astype(bf)
        for b in range(B)
    ]
    in_maps = []
    for core in range(N_CORES):
        b, g = divmod(core, 4)
        hs = slice(g * 256, (g + 1) * 256)
        if core < 4:
            wqkv_f = np.concatenate([Wq[:, hs], Wk[:, hs], Wv[:, hs]], axis=1)
            shards = {
                "wqkv": np.ascontiguousarray(
                    wqkv_f.reshape(8, 128, 768).transpose(1, 0, 2)
                ).astype(bf),
                "wo": np.ascontiguousarray(
                    Wo[hs, :].reshape(2, 128, D).transpose(1, 0, 2)
                ).astype(bf),
            }
        else:
            shards = {k: in_maps[core - 4][k] for k in ("wqkv", "wo")}
        in_maps.append({"xT": xTs[b], "masks": masks, **shards})
    return in_maps


def _run(x, Wq, Wk, Wv, Wo, bo, trace=False):
    from concourse.bass_utils import run_bass_kernel_spmd

    nc = _get_bass()
    in_maps = _make_in_maps(x, Wq, Wk, Wv, Wo)
    res = run_bass_kernel_spmd(
        nc, in_maps, core_ids=list(range(N_CORES)), trace=trace
    )
    out = np.zeros((B, S, D), np.float32)
    for core in range(N_CORES):
        out[core // 4] += res.results[core]["out"].astype(np.float32)
    out += bo.astype(np.float32)
    return out, res


def kernel(x, Wq, Wk, Wv, Wo, bo):
    x, Wq, Wk, Wv, Wo, bo = (np.asarray(a) for a in (x, Wq, Wk, Wv, Wo, bo))
    out, _ = _run(x, Wq, Wk, Wv, Wo, bo, trace=False)
    return out


def kernel_traced(x, Wq, Wk, Wv, Wo, bo):
    """Same as kernel() but returns (out, BassKernelResults) with profiling."""
    x, Wq, Wk, Wv, Wo, bo = (np.asarray(a) for a in (x, Wq, Wk, Wv, Wo, bo))
    return _run(x, Wq, Wk, Wv, Wo, bo, trace=True)


# revision 11
# speedup vs baseline: 1.0476x; 1.0172x over previous
"""Multi-head causal attention (B=2, S=2048, D=1024, H=16, Dh=64) on 8 TRN2 cores.

Sharding: core = (b, g) with b = batch (2), g = head-group (4 heads each).
Each core computes QKV projections for its batch against its 4 heads' weight
columns, causal attention for those heads, and the partial output projection
against its 4 heads' Wo rows.  Host sums the 4 partials per batch and adds
the bias.

Precision: bf16 matmuls with fp32 PSUM accumulation everywhere EXCEPT the
score matmuls, which store Q^T/K^T in fp8 (e4m3) and run in DoubleRow perf
mode: lhsT/rhs carry a stride-0 broadcast pair so one 0.5-cycle/row DR pass
contracts dh=64 twice (the doubled scores fold into the softmax exp scale
1/16).  fp8 elsewhere fails the 2e-2 gate: per-element quantization noise
(~2.7% for e4m3) passes through dot products against random data undamped,
and the independent contributions stack to ~5.5e-2.

Layouts avoid all on-chip transposes:
  x^T [128, 8k, S] k-tile-major feeds projections directly
  V is projected in [s, dh] orientation (x^T tiles as lhsT), landing
  AV-ready with an appended ones column (row 64 accumulates softmax sums)
  scores are computed transposed [k, q] so exp output feeds AV directly

Engine split: PE does matmuls only; ACT does exp only; DVE handles
PSUM-sourced copies/reciprocals and the normalization multiply; the
otherwise-idle GPSIMD does the causal staircase mask multiplies and the
1/sums partition broadcast (replacing the baseline's rank-1 PE matmuls).
"""

import numpy as np
import ml_dtypes

B = 2
S = 2048
D = 1024
HPC = 4  # heads per core
DH = 64
QB = 512  # q band width
NB = S // QB  # 4 bands
KT = 128  # k tile
N_CORES = 8

# exp(s_psum * EXP_SCALE) = exp(s_true / sqrt(DH)); the stride-0 DR pair
# doubles s_psum.
EXP_SCALE = 1.0 / 16.0

_CACHE = {}


def _build_bass():
    import concourse.bacc as bacc
    import concourse.tile as tile
    from concourse import mybir

    f32 = mybir.dt.float32
    bf16 = mybir.dt.bfloat16
    fp8 = mybir.dt.float8e4
    DR = mybir.MatmulPerfMode.DoubleRow
    ExpF = mybir.ActivationFunctionType.Exp

    nc = bacc.Bacc("TRN2", target_bir_lowering=False)

    xT_d = nc.dram_tensor("xT", [128, 8, S], bf16, kind="ExternalInput")
    wqkv_d = nc.dram_tensor("wqkv", [128, 8, 768], bf16, kind="ExternalInput")
    wo_d = nc.dram_tensor("wo", [128, 2, D], bf16, kind="ExternalInput")
    masks_d = nc.dram_tensor("masks", [128, 4, QB], bf16, kind="ExternalInput")
    out_d = nc.dram_tensor("out", [S, D], bf16, kind="ExternalOutput")

    with tile.TileContext(nc) as tc:
        with (
            tc.tile_pool(name="consts", bufs=1) as consts,
            tc.tile_pool(name="persist", bufs=1) as persist,
            tc.tile_pool(name="score_ps", bufs=2, space="PSUM") as score_ps,
            tc.tile_pool(name="ctx_ps", bufs=2, space="PSUM") as ctx_ps,
            tc.tile_pool(name="misc_ps", bufs=2, space="PSUM") as misc_ps,
            tc.tile_pool(name="at_pool", bufs=8) as at_pool,
            tc.tile_pool(name="rr_pool", bufs=4) as rr_pool,
            tc.tile_pool(name="rb_pool", bufs=4) as rb_pool,
            tc.tile_pool(name="osb_pool", bufs=6) as osb_pool,
        ):
            # ---- constants: weights first (first proj group needs them),
            #      band-0 x^T slices, masks; later x^T bands stream behind ----
            wqkv = consts.tile([128, 8, 768], bf16, tag="wqkv", name="wqkv")
            xT = consts.tile([128, 8, S], bf16, tag="xT", name="xT")
            nc.sync.dma_start(out=wqkv[:, 0, :], in_=wqkv_d[:, 0, :])
            nc.sync.dma_start(out=xT[:, 0:4, 0:QB], in_=xT_d[:, 0:4, 0:QB])
            for k in range(1, 4):
                nc.sync.dma_start(out=wqkv[:, k, :], in_=wqkv_d[:, k, :])
            nc.sync.dma_start(out=xT[:, 4:8, 0:QB], in_=xT_d[:, 4:8, 0:QB])
            for k in range(4, 8):
                nc.sync.dma_start(out=wqkv[:, k, :], in_=wqkv_d[:, k, :])
            for j in range(1, NB):
                nc.sync.dma_start(
                    out=xT[:, :, j * QB : (j + 1) * QB],
                    in_=xT_d[:, :, j * QB : (j + 1) * QB],
                )
            mask_sb = consts.tile([128, 4, QB], bf16, tag="masks", name="masks")
            nc.sync.dma_start(out=mask_sb, in_=masks_d[:, :, :])
            wo = consts.tile([128, 2, D], bf16, tag="wo", name="wo")
            nc.sync.dma_start(out=wo, in_=wo_d[:, :, :])

            # ---- persistent activations ----
            qT = [
                persist.tile([128, S], fp8, tag=f"qT{p}", name=f"qT{p}")
                for p in range(2)
            ]
            kTt = [
                persist.tile([128, S], fp8, tag=f"kT{p}", name=f"kT{p}")
                for p in range(2)
            ]
            # v: (k-position, k-tile, head-in-pair, dh + ones column)
            vp = [
                persist.tile([128, 16, 2, 65], bf16, tag=f"vp{p}", name=f"vp{p}")
                for p in range(2)
            ]
            # ctx^T, normalized: (dh-in-pair, pair, q)
            ctxo = persist.tile([128, 2, S], bf16, tag="ctxo", name="ctxo")
            for p in range(2):
                nc.gpsimd.memset(vp[p][:, :, :, 64:65], 1.0)

            def dr2(ap, n):
                """View a [64, n] slice as a stride-0 [64, 2, n] DR pair."""
                return ap.unsqueeze(1).broadcast_to([64, 2, n])

            filler_q = []  # (est_ns, tag, closure) independent PE chains

            def emit_qk_chain(t, dest, p, j, h):
                q0 = j * QB + h * (QB // 2)
                c0 = 256 * t + 128 * p
                ps = misc_ps.tile([128, QB // 2], f32, tag="misc", name="pqk")
                for k in range(8):
                    nc.tensor.matmul(
                        ps,
                        lhsT=wqkv[:, k, c0 : c0 + 128],
                        rhs=xT[:, k, q0 : q0 + QB // 2],
                        start=(k == 0),
                        stop=(k == 7),
                    )
                nc.vector.tensor_copy(
                    out=dest[p][:, q0 : q0 + QB // 2], in_=ps
                )

            def emit_v_chain(kt, p):
                c0 = 512 + 128 * p
                ps = misc_ps.tile([128, 2, 64], f32, tag="misc", name="pv")
                for k in range(8):
                    nc.tensor.matmul(
                        ps,
                        lhsT=xT[:, k, kt * KT : (kt + 1) * KT],
                        rhs=wqkv[:, k, c0 : c0 + 128],
                        start=(k == 0),
                        stop=(k == 7),
                    )
                nc.vector.tensor_copy(out=vp[p][:, kt, :, 0:64], in_=ps)

            def emit_proj(j):
                """QKV projections for band j (bf16, fp32 PSUM).

                Q^T/K^T land as fp8 [128, QB] slabs (pair rows = 2 heads x
                64 dh) feeding the DR score matmuls.  V is projected
                directly in [s, dh] orientation (x^T tiles as lhsT), so no
                on-chip transposes are needed."""
                for t, dest in ((0, qT), (1, kTt)):
                    for p in range(2):
                        for h in range(2):
                            emit_qk_chain(t, dest, p, j, h)
                for kt4 in range(4):
                    for p in range(2):
                        emit_v_chain(4 * j + kt4, p)

            def queue_proj(j):
                for t, dest in ((0, qT), (1, kTt)):
                    for p in range(2):
                        for h in range(2):
                            filler_q.append(
                                (860, ("proj", j),
                                 lambda t=t, dest=dest, p=p, h=h:
                                     emit_qk_chain(t, dest, p, j, h))
                            )
                for kt4 in range(4):
                    for p in range(2):
                        filler_q.append(
                            (430, ("proj", j),
                             lambda kt=4 * j + kt4, p=p: emit_v_chain(kt, p))
                        )

            def emit_op_chain(j, m, n, last):
                NQ = QB // 2
                ops = misc_ps.tile([128, NQ], f32, tag="misc", name="ops")
                for p in range(2):
                    nc.tensor.matmul(
                        ops,
                        lhsT=ctxo[:, p, m * KT : (m + 1) * KT],
                        rhs=wo[:, p, n * NQ : (n + 1) * NQ],
                        start=(p == 0),
                        stop=(p == 1),
                    )
                osb = osb_q[m]
                nc.vector.tensor_copy(out=osb[:, n * NQ : (n + 1) * NQ], in_=ops)
                if n == 3:
                    nc.sync.dma_start(
                        out=out_d[m * KT : (m + 1) * KT, :], in_=osb
                    )

            osb_q = {}

            def queue_outproj(j):
                last = j == NB - 1
                for m in range(4 * j, 4 * j + 4):
                    osb_q[m] = osb_pool.tile([128, D], bf16, tag="osb", name="osb")
                    for n in range(4):
                        filler_q.append(
                            (215, ("outproj", j),
                             lambda m=m, n=n: emit_op_chain(j, m, n, last))
                        )

            bal = {"act": 0.0, "pe": 0.0}

            def drain_fillers(tag_proj_band=None, all_=False):
                """Emit queued chains: FIFO through the last must-emit item
                (band j's projection matmuls must precede band j's score
                matmuls in the in-order PE stream), then keep filling while
                the ACT-time estimate leads the PE one."""

                def is_must(e):
                    kind, b = e[1]
                    return all_ or (
                        kind == "proj"
                        and tag_proj_band is not None
                        and b <= tag_proj_band
                    )

                while any(is_must(e) for e in filler_q):
                    est, _, cl = filler_q.pop(0)
                    cl()
                    bal["pe"] += est
                while filler_q and bal["act"] > bal["pe"]:
                    est, _, cl = filler_q.pop(0)
                    cl()
                    bal["pe"] += est

            deferred_pool = []

            def emit_norm_pc(j, p, c, cps):
                """ctx rows / softmax sums (ctx PSUM row 64): DVE does the
                dependency-free PSUM reads (reciprocal of the sums row, ctx
                copy to SBUF -- releasing the ctx PSUM bank early); GPSIMD
                broadcasts 1/sums across partitions and applies the
                all-SBUF normalization multiply, keeping the cross-engine
                wait off the DVE queue (whose copies release the PE's
                PSUM slots)."""
                q0 = j * QB
                rr = rr_pool.tile([1, QB], bf16, tag="rr", name="rr")
                with nc.allow_low_precision(
                    reason="reciprocal feeds a bf16 multiply"
                ):
                    nc.vector.reciprocal(out=rr, in_=cps[64:65, :])
                cf = rb_pool.tile([64, QB], bf16, tag="cf", name="cf")
                nc.vector.tensor_copy(out=cf, in_=cps[0:64, :])
                def pool_norm():
                    rbs = rb_pool.tile([64, QB], bf16, tag="rb", name="rb")
                    nc.gpsimd.partition_broadcast(rbs, rr)
                    nc.gpsimd.tensor_mul(
                        ctxo[64 * c : 64 * c + 64, p, q0 : q0 + QB],
                        cf,
                        rbs,
                    )

                deferred_pool.append(pool_norm)

            def emit_attention(j):
                """Scores+softmax+AV for band j.

                Scores land transposed ([k, q]) in a [128, 2, QB] fp32 PSUM
                tile per (pair, k-tile-pair, head); one exp covers both
                halves.  Diagonal pairs extend the odd k-tile's q-range down
                to the even tile's start so the exp stays a single strided
                instruction; the AV matmuls read per-tile causal ranges so
                the extension region is never consumed.  GPSIMD applies the
                128-wide staircase mask strips after exp.

                Per (pair, k-pair) group the PE stream is: both heads'
                score matmuls, then queued filler chains sized to the
                ACT-vs-PE balance (the exp is ~3x the group's matmul time),
                then both heads' AV matmuls."""
                q0 = j * QB
                n_i2 = 2 * (j + 1)
                for p in range(2):
                    cps = [
                        ctx_ps.tile([65, QB], f32, tag="ctx", name="ctx")
                        for _ in range(2)
                    ]
                    for i2 in range(n_i2):
                        o_e = 2 * i2 - 4 * j
                        diag = o_e >= 0
                        z_e = 128 * o_e if diag else 0
                        z_o = z_e + 128 if diag else 0
                        sps_c, at_c = [], []
                        for c in range(2):
                            sps = score_ps.tile(
                                [128, 2, QB], f32, tag="sps", name="sps"
                            )
                            for half in range(2):
                                i = 2 * i2 + half
                                nc.tensor.matmul(
                                    sps[:, half, z_e:QB],
                                    lhsT=dr2(
                                        kTt[p][
                                            64 * c : 64 * c + 64,
                                            i * KT : (i + 1) * KT,
                                        ],
                                        KT,
                                    ),
                                    rhs=dr2(
                                        qT[p][
                                            64 * c : 64 * c + 64,
                                            q0 + z_e : q0 + QB,
                                        ],
                                        QB - z_e,
                                    ),
                                    start=True,
                                    stop=True,
                                    perf_mode=DR,
                                )
                            sps_c.append(sps)
                            bal["pe"] += 2 * (QB - z_e) * 0.5 * 0.42
                        for c in range(2):
                            at = at_pool.tile(
                                [128, 2, QB], bf16, tag="at", name="at"
                            )
                            nc.scalar.activation(
                                out=at[:, :, z_e:QB],
                                in_=sps_c[c][:, :, z_e:QB],
                                func=ExpF,
                                scale=EXP_SCALE,
                            )
                            if diag:
                                nc.gpsimd.tensor_mul(
                                    at[:, 0, z_e:z_o],
                                    at[:, 0, z_e:z_o],
                                    mask_sb[:, o_e, z_e:z_o],
                                )
                                nc.gpsimd.tensor_mul(
                                    at[:, 1, z_o : z_o + 128],
                                    at[:, 1, z_o : z_o + 128],
                                    mask_sb[:, o_e + 1, z_o : z_o + 128],
                                )
                            at_c.append(at)
                            bal["act"] += 2 * (QB - z_e) * 0.833 + 185
                        while deferred_pool:
                            deferred_pool.pop(0)()
                        drain_fillers()
                        for c in range(2):
                            for half in range(2):
                                i = 2 * i2 + half
                                o = i - 4 * j
                                z = 128 * o if o > 0 else 0
                                nc.tensor.matmul(
                                    cps[c][:, z:QB],
                                    lhsT=vp[p][:, i, c, :],
                                    rhs=at_c[c][:, half, z:QB],
                                    start=(i == 0),
                                    stop=(i == 4 * (j + 1) - 1),
                                )
                                bal["pe"] += (QB - z) * 0.42
                    for c in range(2):
                        emit_norm_pc(j, p, c, cps[c])

            # band-major pipeline with filler interleaving: the next
            # band's projection chains and the previous band's output
            # projection fill the PE bubbles while exp (ACT) works through
            # the current band's score tiles.
            emit_proj(0)
            for j in range(NB):
                if j + 1 < NB:
                    queue_proj(j + 1)
                emit_attention(j)
                while deferred_pool:
                    deferred_pool.pop(0)()
                drain_fillers(tag_proj_band=j + 1)
                queue_outproj(j)
            drain_fillers(all_=True)

    nc.compile()
    return nc


def _get_bass():
    if "nc" not in _CACHE:
        _CACHE["nc"] = _build_bass()
    return _CACHE["nc"]


def _make_in_maps(x, Wq, Wk, Wv, Wo):
    bf = ml_dtypes.bfloat16
    if "masks" not in _CACHE:
        # causal staircase masks: keep iff q >= k + 128*o  (within a band, a
        # k-tile at offset o*128 above the band start)
        kp = np.arange(128)[:, None]
        qf = np.arange(QB)[None, :]
        _CACHE["masks"] = np.ascontiguousarray(
            np.stack(
                [(qf >= kp + 128 * o).astype(np.float32) for o in range(4)]
            ).transpose(1, 0, 2)
        ).astype(bf)
    masks = _CACHE["masks"]

    # x^T in k-tile-major layout: (p, k, s) = x[b][s, 128k + p]
    xTs = [
        np.ascontiguousarray(
            x[b].T.reshape(8, 128, S).transpose(1, 0, 2)
        ).astype(bf)
        for b in range(B)
    ]
    in_maps = []
    for core in range(N_CORES):
        b, g = divmod(core, 4)
        hs = slice(g * 256, (g + 1) * 256)
        if core < 4:
            wqkv_f = np.concatenate([Wq[:, hs], Wk[:, hs], Wv[:, hs]], axis=1)
            shards = {
                "wqkv": np.ascontiguousarray(
                    wqkv_f.reshape(8, 128, 768).transpose(1, 0, 2)
                ).astype(bf),
                "wo": np.ascontiguousarray(
                    Wo[hs, :].reshape(2, 128, D).transpose(1, 0, 2)
                ).astype(bf),
            }
        else:
            shards = {k: in_maps[core - 4][k] for k in ("wqkv", "wo")}
        in_maps.append({"xT": xTs[b], "masks": masks, **shards})
    return in_maps


def _run(x, Wq, Wk, Wv, Wo, bo, trace=False):
    from concourse.bass_utils import run_bass_kernel_spmd

    nc = _get_bass()
    in_maps = _make_in_maps(x, Wq, Wk, Wv, Wo)
    res = run_bass_kernel_spmd(
        nc, in_maps, core_ids=list(range(N_CORES)), trace=trace
    )
    out = np.zeros((B, S, D), np.float32)
    for core in range(N_CORES):
        out[core // 4] += res.results[core]["out"].astype(np.float32)
    out += bo.astype(np.float32)
    return out, res


def kernel(x, Wq, Wk, Wv, Wo, bo):
    x, Wq, Wk, Wv, Wo, bo = (np.asarray(a) for a in (x, Wq, Wk, Wv, Wo, bo))
    out, _ = _run(x, Wq, Wk, Wv, Wo, bo, trace=False)
    return out


def kernel_traced(x, Wq, Wk, Wv, Wo, bo):
    """Same as kernel() but returns (out, BassKernelResults) with profiling."""
    x, Wq, Wk, Wv, Wo, bo = (np.asarray(a) for a in (x, Wq, Wk, Wv, Wo, bo))
    return _run(x, Wq, Wk, Wv, Wo, bo, trace=True)
